# revision 18
# baseline (speedup 1.0000x reference)
"""Trainium2 Bass kernel for the capsule-routing layer (nn_Caps_Layer).

Full inputs: x [32, 512, 768] f32, W [1, 768, 512] f32.
Output: [32, 16, 32] f32.

Strategy: data-parallel over batch across 8 NeuronCores (4 batches/core).
Host-side prep (free wrt device time): x is pre-transposed to d-major
[768, 512] per batch and cast to bf16; W cast to bf16. This removes all
on-device x transposes and halves HBM traffic.

Per core:
  - u[s, (n c)] tiles via PE bf16 matmuls from xT tiles (no transposes)
  - uT[(n c), s] tiles via PE transposes of u (bf16, 1 cyc/row)
  - 3 routing iterations fully on-chip with narrow (16-wide) matmuls that
    land results directly in the layout the next step needs:
      outputsT tile OFT[nc, n]  = sum_sc u_chunk^T @ cwT       (PE)
      bT[s, n]                  = sum_kc uT_chunk^T @ mblk     (PE)
    softmax over n on the free axis; squash's 1/sqrt via exp(-0.5*ln(x))
    so every ACT func lives in one activation table (exp/ln/copy/square)
    -> a single table load for the whole kernel.
  - final gather via a tiled-identity matrix; DMA out per batch.
"""
import numpy as np
import concourse.bass as bass
import concourse.mybir as mybir
import concourse.tile as tile
from concourse import bacc
from concourse.bass import ts, ds
from concourse.bass_utils import run_bass_kernel_spmd

F32 = mybir.dt.float32
BF16 = mybir.dt.bfloat16
AF = mybir.ActivationFunctionType
AX = mybir.AxisListType
OP = mybir.AluOpType

NCORES = 8
B, S, D = 32, 512, 768
N, C = 16, 32
NC = N * C            # 512
BL = B // NCORES      # 4 batches per core
EPS = 1e-7
SCN = S // 128        # 4 s-chunks
DCN = D // 128        # 6 d-chunks
KCN = NC // 128       # 4 nc-chunks
ROUTINGS = 3
NWARM = 28            # PE warm-up transposes (cover p-state ramp + DMA lead-in)


def _build_module():
    nc = bacc.Bacc("TRN2", target_bir_lowering=False, num_devices=NCORES)
    XT = nc.dram_tensor("xt", [BL, D, S], BF16, kind="ExternalInput")
    W = nc.dram_tensor("w", [D, NC], BF16, kind="ExternalInput")
    CB = nc.dram_tensor("cb", [128, 160], BF16, kind="ExternalInput")
    MK = nc.dram_tensor("mk", [128, 128], F32, kind="ExternalInput")
    OUT = nc.dram_tensor("out", [BL, N, C], F32, kind="ExternalOutput")

    ev_flip = [0]

    with tile.TileContext(nc) as tc:
        with (
            tc.tile_pool(name="const", bufs=1) as pc,
            tc.tile_pool(name="xp", bufs=3) as px_pool,
            tc.tile_pool(name="up", bufs=16) as pu_pool,
            tc.tile_pool(name="utp", bufs=16) as put_pool,
            tc.tile_pool(name="rt", bufs=6) as prt,
            tc.tile_pool(name="mmp", bufs=3, space="PSUM") as pmm,
            tc.tile_pool(name="trp", bufs=2, space="PSUM") as ptr,
            tc.tile_pool(name="rmp", bufs=3, space="PSUM") as prm,
        ):
            def evac(dst, src):
                # PSUM->SBUF evacuations alternate DVE/ACT (GPSIMD can't
                # read PSUM)
                if ev_flip[0] % 2 == 0:
                    nc.vector.tensor_copy(dst, src)
                else:
                    nc.scalar.copy(dst, src)
                ev_flip[0] += 1

            # ---- on-chip constants (no DMA) ----
            warm = pc.tile([128, 128], BF16, tag="warm")
            ones16b = pc.tile([128, 16], BF16, tag="ones16b")
            ones128b = pc.tile([128, 1], BF16, tag="ones128b")
            ones1xb = pc.tile([1, 128], BF16, tag="ones1xb")
            epst = pc.tile([1, 1], F32, tag="eps")
            nc.gpsimd.memset(warm[:], 0.0)
            nc.gpsimd.memset(ones16b[:], 1.0)
            nc.gpsimd.memset(ones128b[:], 1.0)
            nc.gpsimd.memset(ones1xb[:], 1.0)
            nc.gpsimd.memset(epst[:], EPS)

            # Pin the activation table to natural_log_exp_and_others up
            # front: it covers every ACT func this kernel uses (exp, ln,
            # square, copy), so the act-table pass inserts no further
            # (1.3us!) table loads mid-chain.
            from concourse.hw_specs import get_activation_tables
            tabs = list(get_activation_tables(nc.m.arch).keys())
            nle_id = tabs.index("natural_log_exp_and_others")
            nc.scalar.add_instruction(
                mybir.InstLoadActFuncSet(
                    name=nc.get_next_instruction_name(),
                    ins=[],
                    outs=[],
                    act_func_set_id=nle_id,
                )
            )

            # PE warm-up: back-to-back dummy transposes keep the tensor
            # engine continuously busy through its p-state ramp while the
            # first x/W chunks stream in.
            wpsum = ptr.tile([128, S], BF16, tag="tr", name="wpsum")
            for _ in range(NWARM):
                nc.tensor.transpose(wpsum[:, 0:128], warm[:], warm[:])

            # ---- DMA'd constants ----
            identb = pc.tile([128, 128], BF16, tag="identb")
            gmatb = pc.tile([128, 32], BF16, tag="gmatb")
            masks = pc.tile([128, 128], F32, tag="masks")
            wsb = pc.tile([128, DCN, NC], BF16, tag="w")

            # ---- stage A: u and uT per batch ----
            # b0's x and W stream in 2-dc chunks, interleaved on one queue so
            # the first matmuls can start after ~2 chunks land.
            us = [[None] * SCN for _ in range(BL)]
            uts = [[None] * KCN for _ in range(BL)]
            xts = []
            for b in range(BL):
                xts.append(px_pool.tile([128, DCN, S], BF16, tag="x",
                                        name=f"xt{b}"))
            for h in range(3):
                nc.sync.dma_start(
                    wsb[:, 2 * h:2 * h + 2, :],
                    W[ds(256 * h, 256), :].rearrange("(dc p) n -> p dc n", p=128),
                )
                nc.sync.dma_start(
                    xts[0][:, 2 * h:2 * h + 2, :],
                    XT[0, ds(256 * h, 256), :].rearrange("(dc p) s -> p dc s", p=128),
                )
            nc.sync.dma_start(identb[:], CB[:, 0:128])
            nc.sync.dma_start(gmatb[:], CB[:, 128:160])
            nc.sync.dma_start(masks[:], MK[:, :])
            for b in range(1, BL):
                nc.sync.dma_start(
                    xts[b][:], XT[b, :, :].rearrange("(dc p) s -> p dc s", p=128)
                )

            def stage_a_mm(b):
                # u tiles [128(s), 512(nc)]: accumulate over dc in dc-major
                # order so b0 consumes x/W chunks as they arrive
                for h in range(2):
                    scs = (2 * h, 2 * h + 1)
                    pus = {sc: pmm.tile([128, NC], F32, tag="mm",
                                        name=f"pu{b}_{sc}")
                           for sc in scs}
                    for dc in range(DCN):
                        for sc in scs:
                            nc.tensor.matmul(
                                pus[sc][:],
                                xts[b][:, dc, ts(sc, 128)],
                                wsb[:, dc, :],
                                start=(dc == 0),
                                stop=(dc == DCN - 1),
                            )
                        pump(1)
                    for sc in scs:
                        u = pu_pool.tile([128, NC], BF16, tag="u",
                                         name=f"u{b}_{sc}")
                        evac(u[:], pus[sc][:])
                        us[b][sc] = u

            def stage_a_ut(b):
                # uT tiles [128(nc), 512(s)] via PE transposes (bf16)
                for kc in range(KCN):
                    put = ptr.tile([128, S], BF16, tag="tr", name=f"put{b}_{kc}")
                    for sc in range(SCN):
                        nc.tensor.transpose(
                            put[:, ts(sc, 128)],
                            us[b][sc][:, ts(kc, 128)],
                            identb[:],
                        )
                    ut = put_pool.tile([128, S], BF16, tag="ut", name=f"ut{b}_{kc}")
                    evac(ut[:], put[:])
                    uts[b][kc] = ut
                    pump(1)

            # ---- routing ----
            # Batch-PAIR chains: one chain handles two batches (vector ops
            # 128 wide), halving chain count and semaphore traffic. Each
            # chain gets ONE PSUM bank, all pieces in DISJOINT column
            # regions (no write-after-read edges):
            #   pot 0:128 | pbt/pf 128:256 | pinv 256:384 | pnsq row0 384:512
            # Chains are emitted as thunk lists and pumped round-robin so
            # each engine's in-order stream matches readiness order.
            cwTs = [None, None]   # per pair
            active = []

            def make_chain(pair, it):
                b0 = 2 * pair
                last = it == ROUTINGS - 1
                st = {}

                def t_oft():
                    work = prm.tile([128, 512], F32, tag="w",
                                    name=f"w{it}_{pair}")
                    st["work"] = work
                    pot = work[:, 0:128]
                    st["pot"] = pot
                    rhs_full = ones16b if it == 0 else cwTs[pair]
                    for bl in range(2):
                        for kc in range(KCN):
                            for sc in range(SCN):
                                rhs = (rhs_full[:, 0:16] if it == 0
                                       else rhs_full[:, ds(bl * 64 + sc * 16, 16)])
                                nc.tensor.matmul(
                                    pot[:, ds(bl * 64 + kc * 16, 16)],
                                    us[b0 + bl][sc][:, ts(kc, 128)],
                                    rhs,
                                    start=(sc == 0),
                                    stop=(sc == SCN - 1),
                                )

                def t_mraw():
                    mraw = prt.tile([128, 128], F32, tag="mraw",
                                    name=f"mraw{it}_{pair}")
                    st["mraw"] = mraw
                    nc.vector.tensor_mul(mraw[:], st["pot"], masks[:])

                def t_sq():
                    sq = prt.tile([128, 128], BF16, tag="sq",
                                  name=f"sq{it}_{pair}")
                    st["sq"] = sq
                    nc.vector.tensor_mul(sq[:], st["mraw"][:], st["mraw"][:])

                def t_nsq():
                    pnsq = st["work"][0:1, ds(384, 128)]
                    st["pnsq"] = pnsq
                    nc.tensor.matmul(pnsq, ones128b[:], st["sq"][:],
                                     start=True, stop=True)

                def t_ln():
                    # capsule norms ride in their owning kc column; dead
                    # columns (masked zeros) turn into eps^-0.5 and multiply
                    # zeros. 1/sqrt(v+eps) = exp(-0.5*ln(v+eps)) keeps every
                    # ACT func in one table.
                    lnt = prt.tile([1, 128], F32, tag="lnt",
                                   name=f"lnt{it}_{pair}")
                    st["lnt"] = lnt
                    nc.scalar.activation(lnt[:], st["pnsq"], AF.Ln,
                                         bias=epst[:])

                def t_exp():
                    invn = prt.tile([1, 128], BF16, tag="invn",
                                    name=f"invn{it}_{pair}")
                    st["invn"] = invn
                    nc.scalar.activation(invn[:], st["lnt"][:], AF.Exp,
                                         scale=-0.5)

                def t_pinv():
                    pinv = st["work"][:, ds(256, 128)]
                    st["pinv"] = pinv
                    nc.tensor.matmul(pinv, ones1xb[:], st["invn"][:],
                                     start=True, stop=True)

                def t_mblk():
                    mblk = prt.tile([128, 128], BF16, tag="mblk",
                                    name=f"mblk{it}_{pair}")
                    st["mblk"] = mblk
                    nc.vector.tensor_mul(mblk[:], st["mraw"][:], st["pinv"])

                def t_pbt():
                    pbt = st["work"][:, ds(128, 128)]
                    st["pbt"] = pbt
                    for bl in range(2):
                        for sc in range(SCN):
                            for kc in range(KCN):
                                nc.tensor.matmul(
                                    pbt[:, ds(bl * 64 + sc * 16, 16)],
                                    uts[b0 + bl][kc][:, ts(sc, 128)],
                                    st["mblk"][:, ds(bl * 64 + kc * 16, 16)],
                                    start=(kc == 0),
                                    stop=(kc == KCN - 1),
                                )

                def t_expb():
                    expb = prt.tile([128, 128], F32, tag="expb",
                                    name=f"expb{it}_{pair}")
                    st["expb"] = expb
                    nc.scalar.activation(expb[:], st["pbt"], AF.Exp)

                def t_zsum():
                    zsum = prt.tile([128, 8], F32, tag="zsum",
                                    name=f"zsum{it}_{pair}")
                    st["zsum"] = zsum
                    nc.vector.tensor_reduce(
                        zsum[:],
                        st["expb"][:].rearrange("p (g n) -> p g n", g=8),
                        axis=AX.X,
                        op=OP.add,
                    )

                def t_zrec():
                    zrec = prt.tile([128, 8], F32, tag="zrec",
                                    name=f"zrec{it}_{pair}")
                    st["zrec"] = zrec
                    nc.vector.reciprocal(zrec[:], st["zsum"][:])

                def t_cwt():
                    zr_ap = st["zrec"][:, :]
                    zr_b = bass.AP(
                        tensor=zr_ap.tensor,
                        offset=zr_ap.offset,
                        ap=[zr_ap.ap[0], [1, 8], [0, 16]],
                    )
                    cwT = prt.tile([128, 128], BF16, tag="cw",
                                   name=f"cw{it}_{pair}")
                    nc.vector.tensor_mul(
                        cwT[:].rearrange("p (g n) -> p g n", g=8),
                        st["expb"][:].rearrange("p (g n) -> p g n", g=8),
                        zr_b,
                    )
                    cwTs[pair] = cwT
                    active.append(make_chain(pair, it + 1))

                def t_gather():
                    for bl in range(2):
                        pf = st["work"][0:16, ds(128 + 32 * bl, 32)]
                        st[f"pf{bl}"] = pf
                        for kc in range(KCN):
                            nc.tensor.matmul(
                                pf,
                                st["mblk"][:, ds(bl * 64 + kc * 16, 16)],
                                gmatb[:],
                                start=(kc == 0),
                                stop=(kc == KCN - 1),
                            )

                def t_out():
                    for bl in range(2):
                        fsb = prt.tile([16, 32], F32, tag="fin",
                                       name=f"fin{pair}_{bl}")
                        nc.vector.tensor_copy(fsb[:], st[f"pf{bl}"])
                        nc.sync.dma_start(OUT[b0 + bl, :, :], fsb[:])

                ops = [t_oft, t_mraw, t_sq, t_nsq, t_ln, t_exp, t_pinv,
                       t_mblk]
                if not last:
                    ops += [t_pbt, t_expb, t_zsum, t_zrec, t_cwt]
                else:
                    ops += [t_gather, t_out]
                return ops

            def pump(n):
                # advance every live chain by up to n ops, round-robin
                for _ in range(n):
                    for q in list(active):
                        if q:
                            q.pop(0)()
                        if not q:
                            active.remove(q)

            def drain():
                while active:
                    pump(1)

            stage_a_mm(0)
            stage_a_ut(0)
            stage_a_mm(1)
            active.append(make_chain(0, 0))
            stage_a_ut(1)
            stage_a_mm(2)
            stage_a_ut(2)
            stage_a_mm(3)
            active.append(make_chain(1, 0))
            stage_a_ut(3)
            drain()

    nc.compile()
    return nc


def _make_consts():
    import ml_dtypes
    bf = ml_dtypes.bfloat16
    cb = np.zeros((128, 160), dtype=bf)
    cb[:, 0:128] = np.eye(128, dtype=np.float32).astype(bf)
    cb[:, 128:160] = np.tile(np.eye(32, dtype=np.float32), (4, 1)).astype(bf)
    masks = np.zeros((128, 64), dtype=np.float32)
    for k in range(4):
        for g in range(4):
            n = 4 * k + g
            masks[32 * g:32 * (g + 1), 16 * k + n] = 1.0
    masks2 = np.concatenate([masks, masks], axis=1)
    return {"cb": cb, "mk": masks2}


_NC_CACHE = []


def kernel(x: np.ndarray, W: np.ndarray) -> np.ndarray:
    import ml_dtypes
    bf = ml_dtypes.bfloat16
    assert x.shape == (B, S, D) and W.shape == (1, D, NC)
    if not _NC_CACHE:
        _NC_CACHE.append(_build_module())
    nc = _NC_CACHE[0]
    consts = _make_consts()
    w2 = np.ascontiguousarray(W[0]).astype(bf)
    in_maps = []
    for i in range(NCORES):
        m = dict(consts)
        xs = x[i * BL:(i + 1) * BL]
        m["xt"] = np.ascontiguousarray(xs.transpose(0, 2, 1)).astype(bf)
        m["w"] = w2
        in_maps.append(m)
    res = run_bass_kernel_spmd(nc, in_maps, list(range(NCORES)))
    out = np.concatenate([res.results[i]["out"] for i in range(NCORES)], axis=0)
    return out.astype(np.float32)


# revision 19
# speedup vs baseline: 1.0336x; 1.0336x over previous
"""Trainium2 Bass kernel for the capsule-routing layer (nn_Caps_Layer).

Full inputs: x [32, 512, 768] f32, W [1, 768, 512] f32.
Output: [32, 16, 32] f32.

Strategy: data-parallel over batch across 8 NeuronCores (4 batches/core).
Host-side prep (free wrt device time): x is pre-transposed to d-major
[768, 512] per batch and cast to bf16; W cast to bf16. This removes all
on-device x transposes and halves HBM traffic.

Per core:
  - u[s, (n c)] tiles via PE bf16 matmuls from xT tiles (no transposes)
  - uT[(n c), s] tiles via PE transposes of u (bf16, 1 cyc/row)
  - 3 routing iterations fully on-chip with narrow (16-wide) matmuls that
    land results directly in the layout the next step needs:
      outputsT tile OFT[nc, n]  = sum_sc u_chunk^T @ cwT       (PE)
      bT[s, n]                  = sum_kc uT_chunk^T @ mblk     (PE)
    softmax over n on the free axis; squash's 1/sqrt via exp(-0.5*ln(x))
    so every ACT func lives in one activation table (exp/ln/copy/square)
    -> a single table load for the whole kernel.
  - final gather via a tiled-identity matrix; DMA out per batch.
"""
import numpy as np
import concourse.bass as bass
import concourse.mybir as mybir
import concourse.tile as tile
from concourse import bacc
from concourse.bass import ts, ds
from concourse.bass_utils import run_bass_kernel_spmd

F32 = mybir.dt.float32
BF16 = mybir.dt.bfloat16
AF = mybir.ActivationFunctionType
AX = mybir.AxisListType
OP = mybir.AluOpType

NCORES = 8
B, S, D = 32, 512, 768
N, C = 16, 32
NC = N * C            # 512
BL = B // NCORES      # 4 batches per core
EPS = 1e-7
SCN = S // 128        # 4 s-chunks
DCN = D // 128        # 6 d-chunks
KCN = NC // 128       # 4 nc-chunks
ROUTINGS = 3
NWARM = 14            # PE warm-up transposes (cover p-state ramp + DMA lead-in)


def _build_module():
    nc = bacc.Bacc("TRN2", target_bir_lowering=False, num_devices=NCORES)
    XT = nc.dram_tensor("xt", [BL, D, S], BF16, kind="ExternalInput")
    W = nc.dram_tensor("w", [D, NC], BF16, kind="ExternalInput")
    CB = nc.dram_tensor("cb", [128, 160], BF16, kind="ExternalInput")
    MK = nc.dram_tensor("mk", [128, 128], F32, kind="ExternalInput")
    OUT = nc.dram_tensor("out", [BL, N, C], F32, kind="ExternalOutput")

    ev_flip = [0]

    with tile.TileContext(nc) as tc:
        with (
            tc.tile_pool(name="const", bufs=1) as pc,
            tc.tile_pool(name="xp", bufs=3) as px_pool,
            tc.tile_pool(name="up", bufs=16) as pu_pool,
            tc.tile_pool(name="utp", bufs=16) as put_pool,
            tc.tile_pool(name="rt", bufs=6) as prt,
            tc.tile_pool(name="mmp", bufs=3, space="PSUM") as pmm,
            tc.tile_pool(name="trp", bufs=2, space="PSUM") as ptr,
            tc.tile_pool(name="rmp", bufs=3, space="PSUM") as prm,
        ):
            def evac(dst, src):
                # PSUM->SBUF evacuations alternate DVE/ACT (GPSIMD can't
                # read PSUM)
                if ev_flip[0] % 2 == 0:
                    nc.vector.tensor_copy(dst, src)
                else:
                    nc.scalar.copy(dst, src)
                ev_flip[0] += 1

            # ---- on-chip constants (no DMA) ----
            warm = pc.tile([128, 128], BF16, tag="warm")
            ones16b = pc.tile([128, 16], BF16, tag="ones16b")
            ones128b = pc.tile([128, 1], BF16, tag="ones128b")
            ones1xb = pc.tile([1, 128], BF16, tag="ones1xb")
            epst = pc.tile([1, 1], F32, tag="eps")
            nc.gpsimd.memset(warm[:], 0.0)
            nc.gpsimd.memset(ones16b[:], 1.0)
            nc.gpsimd.memset(ones128b[:], 1.0)
            nc.gpsimd.memset(ones1xb[:], 1.0)
            nc.gpsimd.memset(epst[:], EPS)

            # Pin the activation table to natural_log_exp_and_others up
            # front: it covers every ACT func this kernel uses (exp, ln,
            # square, copy), so the act-table pass inserts no further
            # (1.3us!) table loads mid-chain.
            from concourse.hw_specs import get_activation_tables
            tabs = list(get_activation_tables(nc.m.arch).keys())
            nle_id = tabs.index("natural_log_exp_and_others")
            nc.scalar.add_instruction(
                mybir.InstLoadActFuncSet(
                    name=nc.get_next_instruction_name(),
                    ins=[],
                    outs=[],
                    act_func_set_id=nle_id,
                )
            )

            # PE warm-up: back-to-back dummy transposes keep the tensor
            # engine continuously busy through its p-state ramp while the
            # first x/W chunks stream in.
            wpsum = ptr.tile([128, S], BF16, tag="tr", name="wpsum")
            for _ in range(NWARM):
                nc.tensor.transpose(wpsum[:, 0:128], warm[:], warm[:])

            # ---- DMA'd constants ----
            identb = pc.tile([128, 128], BF16, tag="identb")
            gmatb = pc.tile([128, 32], BF16, tag="gmatb")
            masks = pc.tile([128, 128], F32, tag="masks")
            wsb = pc.tile([128, DCN, NC], BF16, tag="w")

            # ---- stage A: u and uT per batch ----
            # b0's x and W stream in 2-dc chunks, interleaved on one queue so
            # the first matmuls can start after ~2 chunks land.
            us = [[None] * SCN for _ in range(BL)]
            uts = [[None] * KCN for _ in range(BL)]
            xts = []
            for b in range(BL):
                xts.append(px_pool.tile([128, DCN, S], BF16, tag="x",
                                        name=f"xt{b}"))
            for h in range(3):
                nc.sync.dma_start(
                    wsb[:, 2 * h:2 * h + 2, :],
                    W[ds(256 * h, 256), :].rearrange("(dc p) n -> p dc n", p=128),
                )
                nc.sync.dma_start(
                    xts[0][:, 2 * h:2 * h + 2, :],
                    XT[0, ds(256 * h, 256), :].rearrange("(dc p) s -> p dc s", p=128),
                )
            nc.sync.dma_start(identb[:], CB[:, 0:128])
            nc.sync.dma_start(gmatb[:], CB[:, 128:160])
            nc.sync.dma_start(masks[:], MK[:, :])
            for b in range(1, BL):
                nc.sync.dma_start(
                    xts[b][:], XT[b, :, :].rearrange("(dc p) s -> p dc s", p=128)
                )

            def stage_a_mm(b):
                # u tiles [128(s), 512(nc)]: accumulate over dc in dc-major
                # order so b0 consumes x/W chunks as they arrive
                for h in range(2):
                    scs = (2 * h, 2 * h + 1)
                    pus = {sc: pmm.tile([128, NC], F32, tag="mm",
                                        name=f"pu{b}_{sc}")
                           for sc in scs}
                    for dc in range(DCN):
                        for sc in scs:
                            nc.tensor.matmul(
                                pus[sc][:],
                                xts[b][:, dc, ts(sc, 128)],
                                wsb[:, dc, :],
                                start=(dc == 0),
                                stop=(dc == DCN - 1),
                            )
                        pump(1)
                    for sc in scs:
                        u = pu_pool.tile([128, NC], BF16, tag="u",
                                         name=f"u{b}_{sc}")
                        evac(u[:], pus[sc][:])
                        us[b][sc] = u

            def stage_a_ut(b):
                # uT tiles [128(nc), 512(s)] via PE transposes (bf16)
                for kc in range(KCN):
                    put = ptr.tile([128, S], BF16, tag="tr", name=f"put{b}_{kc}")
                    for sc in range(SCN):
                        nc.tensor.transpose(
                            put[:, ts(sc, 128)],
                            us[b][sc][:, ts(kc, 128)],
                            identb[:],
                        )
                    ut = put_pool.tile([128, S], BF16, tag="ut", name=f"ut{b}_{kc}")
                    evac(ut[:], put[:])
                    uts[b][kc] = ut
                    pump(1)

            # ---- routing ----
            # Single-batch chains emitted as thunk lists, pumped round-robin
            # (software pipelining) so concurrent chains interleave in each
            # engine's in-order stream. Each chain gets ONE PSUM bank with
            # all pieces in DISJOINT column regions:
            #   pot 0:64 | pbt 64:128 | pinv 128:192 | pnsq row0 192:256
            #   pf [0:16, 256:288]
            cwTs = [None] * BL
            active = []

            def make_chain(b, it):
                last = it == ROUTINGS - 1
                st = {}

                def t_oft():
                    work = prm.tile([128, 512], F32, tag="w",
                                    name=f"w{it}_{b}")
                    st["work"] = work
                    pot = work[:, 0:64]
                    st["pot"] = pot
                    rhs_full = ones16b if it == 0 else cwTs[b]
                    for kc in range(KCN):
                        for sc in range(SCN):
                            rhs = (rhs_full[:, 0:16] if it == 0
                                   else rhs_full[:, ts(sc, 16)])
                            nc.tensor.matmul(
                                pot[:, ts(kc, 16)],
                                us[b][sc][:, ts(kc, 128)],
                                rhs,
                                start=(sc == 0),
                                stop=(sc == SCN - 1),
                            )

                def t_mraw():
                    mraw = prt.tile([128, 64], F32, tag="mraw",
                                    name=f"mraw{it}_{b}")
                    st["mraw"] = mraw
                    nc.vector.tensor_mul(mraw[:], st["pot"], masks[:, 0:64])

                def t_sq():
                    sq = prt.tile([128, 64], BF16, tag="sq", name=f"sq{it}_{b}")
                    st["sq"] = sq
                    nc.vector.tensor_mul(sq[:], st["mraw"][:], st["mraw"][:])

                def t_nsq():
                    pnsq = st["work"][0:1, ds(192, 64)]
                    st["pnsq"] = pnsq
                    nc.tensor.matmul(pnsq, ones128b[:], st["sq"][:],
                                     start=True, stop=True)

                def t_ln():
                    # capsule norms ride in their owning kc column; dead
                    # columns (masked zeros) become eps^-0.5 and multiply
                    # zeros. 1/sqrt(v+eps) = exp(-0.5*ln(v+eps)) keeps every
                    # ACT func in one table.
                    lnt = prt.tile([1, 64], F32, tag="lnt", name=f"lnt{it}_{b}")
                    st["lnt"] = lnt
                    nc.scalar.activation(lnt[:], st["pnsq"], AF.Ln, bias=epst[:])

                def t_exp():
                    invn = prt.tile([1, 64], BF16, tag="invn",
                                    name=f"invn{it}_{b}")
                    st["invn"] = invn
                    nc.scalar.activation(invn[:], st["lnt"][:], AF.Exp,
                                         scale=-0.5)

                def t_pinv():
                    pinv = st["work"][:, ds(128, 64)]
                    st["pinv"] = pinv
                    nc.tensor.matmul(pinv, ones1xb[:], st["invn"][:],
                                     start=True, stop=True)

                def t_mblk():
                    mblk = prt.tile([128, 64], BF16, tag="mblk",
                                    name=f"mblk{it}_{b}")
                    st["mblk"] = mblk
                    nc.vector.tensor_mul(mblk[:], st["mraw"][:], st["pinv"])

                def t_pbt():
                    pbt = st["work"][:, ds(64, 64)]
                    st["pbt"] = pbt
                    for sc in range(SCN):
                        for kc in range(KCN):
                            nc.tensor.matmul(
                                pbt[:, ts(sc, 16)],
                                uts[b][kc][:, ts(sc, 128)],
                                st["mblk"][:, ts(kc, 16)],
                                start=(kc == 0),
                                stop=(kc == KCN - 1),
                            )

                def t_expb():
                    expb = prt.tile([128, 64], F32, tag="expb",
                                    name=f"expb{it}_{b}")
                    st["expb"] = expb
                    nc.scalar.activation(expb[:], st["pbt"], AF.Exp)

                def t_zsum():
                    zsum = prt.tile([128, 4], F32, tag="zsum",
                                    name=f"zsum{it}_{b}")
                    st["zsum"] = zsum
                    nc.vector.tensor_reduce(
                        zsum[:],
                        st["expb"][:].rearrange("p (g n) -> p g n", g=4),
                        axis=AX.X,
                        op=OP.add,
                    )

                def t_zrec():
                    zrec = prt.tile([128, 4], F32, tag="zrec",
                                    name=f"zrec{it}_{b}")
                    st["zrec"] = zrec
                    nc.vector.reciprocal(zrec[:], st["zsum"][:])

                def t_cwt():
                    zr_ap = st["zrec"][:, :]
                    zr_b = bass.AP(
                        tensor=zr_ap.tensor,
                        offset=zr_ap.offset,
                        ap=[zr_ap.ap[0], [1, 4], [0, 16]],
                    )
                    cwT = prt.tile([128, 64], BF16, tag="cw", name=f"cw{it}_{b}")
                    nc.vector.tensor_mul(
                        cwT[:].rearrange("p (g n) -> p g n", g=4),
                        st["expb"][:].rearrange("p (g n) -> p g n", g=4),
                        zr_b,
                    )
                    cwTs[b] = cwT
                    active.append(make_chain(b, it + 1))

                def t_gather():
                    pf = st["work"][0:16, ds(256, 32)]
                    st["pf"] = pf
                    for kc in range(KCN):
                        nc.tensor.matmul(
                            pf,
                            st["mblk"][:, ts(kc, 16)],
                            gmatb[:],
                            start=(kc == 0),
                            stop=(kc == KCN - 1),
                        )

                def t_out():
                    fsb = prt.tile([16, 32], F32, tag="fin", name=f"fin{b}")
                    nc.vector.tensor_copy(fsb[:], st["pf"])
                    nc.sync.dma_start(OUT[b, :, :], fsb[:])

                ops = [t_oft, t_mraw, t_sq, t_nsq, t_ln, t_exp, t_pinv,
                       t_mblk]
                if not last:
                    ops += [t_pbt, t_expb, t_zsum, t_zrec, t_cwt]
                else:
                    ops += [t_gather, t_out]
                return ops

            def pump(n):
                # advance every live chain by up to n ops, round-robin
                for _ in range(n):
                    for q in list(active):
                        if q:
                            q.pop(0)()
                        if not q:
                            active.remove(q)

            def drain():
                while active:
                    pump(1)

            stage_a_mm(0)
            active.append(make_chain(0, 0))
            stage_a_ut(0)
            stage_a_mm(1)
            active.append(make_chain(1, 0))
            stage_a_ut(1)
            stage_a_mm(2)
            active.append(make_chain(2, 0))
            stage_a_ut(2)
            stage_a_mm(3)
            active.append(make_chain(3, 0))
            stage_a_ut(3)
            drain()

    nc.compile()
    return nc


def _make_consts():
    import ml_dtypes
    bf = ml_dtypes.bfloat16
    cb = np.zeros((128, 160), dtype=bf)
    cb[:, 0:128] = np.eye(128, dtype=np.float32).astype(bf)
    cb[:, 128:160] = np.tile(np.eye(32, dtype=np.float32), (4, 1)).astype(bf)
    masks = np.zeros((128, 64), dtype=np.float32)
    for k in range(4):
        for g in range(4):
            n = 4 * k + g
            masks[32 * g:32 * (g + 1), 16 * k + n] = 1.0
    masks2 = np.concatenate([masks, masks], axis=1)
    return {"cb": cb, "mk": masks2}


_NC_CACHE = []


def kernel(x: np.ndarray, W: np.ndarray) -> np.ndarray:
    import ml_dtypes
    bf = ml_dtypes.bfloat16
    assert x.shape == (B, S, D) and W.shape == (1, D, NC)
    if not _NC_CACHE:
        _NC_CACHE.append(_build_module())
    nc = _NC_CACHE[0]
    consts = _make_consts()
    w2 = np.ascontiguousarray(W[0]).astype(bf)
    in_maps = []
    for i in range(NCORES):
        m = dict(consts)
        xs = x[i * BL:(i + 1) * BL]
        m["xt"] = np.ascontiguousarray(xs.transpose(0, 2, 1)).astype(bf)
        m["w"] = w2
        in_maps.append(m)
    res = run_bass_kernel_spmd(nc, in_maps, list(range(NCORES)))
    out = np.concatenate([res.results[i]["out"] for i in range(NCORES)], axis=0)
    return out.astype(np.float32)
